# revision 2
# baseline (speedup 1.0000x reference)
"""Trainium2 Bass kernel for CustomMultiheadAttention.

Shapes: query/key_/value [S=2048, B=2, E=1024] f32, H=16 heads, D=64.
Returns (out [S,B,E], attn [B,H,S,S]) like the torch/jax reference.

Sharding: tensor-parallel over heads — each of the 8 NeuronCores computes
2 heads' worth of Q/K/V projections, full attention for its (batch, head)
pairs, the attention-probability output slice, and a partial output
projection (reduced on host).

Per-core device program (all f32):
  - Q,K projected into "Y^T" layout [d_local=128, tokens=4096] (tokens
    b-major), with 1/sqrt(D) folded into Wq/bq on host.
  - V projected into natural layout [tokens, 128].
  - scores[q,k] for one 128-row q-block: PSUM [128, 2048] built as
    identity@bias (PSUM preload) + Q_h^T.T @ K_h^T accumulate.
  - One Exp activation per q-block gives probabilities AND the softmax
    denominator via accum_out; DVE normalizes in place; DMA writes the
    attn slice (contiguous rows).
  - Context: PE-transpose of attn tiles -> p^T, then p^T.T @ V accumulated
    in PSUM; transposed context feeds the partial out-projection.
"""

import numpy as np

S, B, E, H = 2048, 2, 1024, 16
D = E // H  # 64
N_CORES = 8
HC = H // N_CORES  # heads per core = 2
DLOC = HC * D  # 128
T = S * B  # 4096 tokens, b-major (t = b*S + s)

F32 = np.float32

_CACHE = {}


def _build_nc():
    import concourse.bacc as bacc
    import concourse.mybir as mybir
    import concourse.tile as tile
    from concourse.masks import make_identity

    dt = mybir.dt.float32
    AF = mybir.ActivationFunctionType

    nc = bacc.Bacc(None, target_bir_lowering=False, debug=False)

    xt_q = nc.dram_tensor("xt_q", [E, T], dt, kind="ExternalInput")
    xt_k = nc.dram_tensor("xt_k", [E, T], dt, kind="ExternalInput")
    xt_v = nc.dram_tensor("xt_v", [E, T], dt, kind="ExternalInput")
    wt_q = nc.dram_tensor("wt_q", [E, DLOC], dt, kind="ExternalInput")
    wt_k = nc.dram_tensor("wt_k", [E, DLOC], dt, kind="ExternalInput")
    wt_v = nc.dram_tensor("wt_v", [E, DLOC], dt, kind="ExternalInput")
    wot = nc.dram_tensor("wot", [DLOC, E], dt, kind="ExternalInput")
    b_q = nc.dram_tensor("b_q", [DLOC, 1], dt, kind="ExternalInput")
    b_k = nc.dram_tensor("b_k", [DLOC, 1], dt, kind="ExternalInput")
    b_v = nc.dram_tensor("b_v", [1, DLOC], dt, kind="ExternalInput")
    bias_m = nc.dram_tensor("bias_m", [S, S], dt, kind="ExternalInput")

    attn_out = nc.dram_tensor("attn_out", [B, HC, S, S], dt, kind="ExternalOutput")
    out_part = nc.dram_tensor("out_part", [T, E], dt, kind="ExternalOutput")

    NQB = S // 128  # 16 q-blocks per batch
    NKT = S // 128  # 16 k tiles per batch
    NKN = S // 512  # 4 wide k chunks

    with tile.TileContext(nc) as tc:
        with (
            tc.tile_pool(name="const", bufs=1) as const,
            tc.tile_pool(name="wpool", bufs=1) as wpool,
            tc.tile_pool(name="xpool", bufs=2) as xpool,
            tc.tile_pool(name="act", bufs=1) as act,
            tc.tile_pool(name="biasp", bufs=2) as biasp,
            tc.tile_pool(name="attnp", bufs=3) as attnp,
            tc.tile_pool(name="ptp", bufs=3) as ptp,
            tc.tile_pool(name="ctxp", bufs=2) as ctxp,
            tc.tile_pool(name="denp", bufs=6) as denp,
            tc.tile_pool(name="outp", bufs=2) as outp,
            tc.tile_pool(name="ps_big", bufs=1, space="PSUM") as ps_big,
            tc.tile_pool(name="ps_med", bufs=2, space="PSUM") as ps_med,
            tc.tile_pool(name="ps_ctx", bufs=2, space="PSUM") as ps_ctx,
        ):
            # ---- constants ----
            ident = const.tile([128, 128], dt)
            make_identity(nc, ident[:])
            ones_row = const.tile([1, 128], dt)
            nc.vector.memset(ones_row[:], 1.0)

            # ---- weights ----
            wq_sb = wpool.tile([128, 8, DLOC], dt)
            wk_sb = wpool.tile([128, 8, DLOC], dt)
            wv_sb = wpool.tile([128, 8, DLOC], dt)
            nc.sync.dma_start(wq_sb[:], wt_q[:, :].rearrange("(a p) m -> p a m", p=128))
            nc.sync.dma_start(wk_sb[:], wt_k[:, :].rearrange("(a p) m -> p a m", p=128))
            nc.sync.dma_start(wv_sb[:], wt_v[:, :].rearrange("(a p) m -> p a m", p=128))
            wot_sb = wpool.tile([128, E], dt)
            nc.sync.dma_start(wot_sb[:], wot[:, :])
            bq_sb = wpool.tile([DLOC, 1], dt)
            bk_sb = wpool.tile([DLOC, 1], dt)
            bv_sb = wpool.tile([1, DLOC], dt)
            nc.sync.dma_start(bq_sb[:], b_q[:, :])
            nc.sync.dma_start(bk_sb[:], b_k[:, :])
            nc.sync.dma_start(bv_sb[:], b_v[:, :])

            # ---- persistent activations ----
            qt_sb = act.tile([128, T], dt)  # Q^T  [dloc, token]
            kt_sb = act.tile([128, T], dt)  # K^T  [dloc, token]
            v_sb = act.tile([128, T // 128, DLOC], dt)  # V [token, dloc]
            ctxT0 = act.tile([128, S], dt)  # context^T for batch 0
            ctxT1 = act.tile([128, S], dt)
            ctxT = [ctxT0, ctxT1]

            # ---- Q, K projections (Y^T layout) ----
            for name, xt, w_sb, b_sb, y_sb in (
                ("q", xt_q, wq_sb, bq_sb, qt_sb),
                ("k", xt_k, wk_sb, bk_sb, kt_sb),
            ):
                for nt in range(T // 512):
                    x_t = xpool.tile([128, 8, 512], dt, tag="xt")
                    nc.sync.dma_start(
                        x_t[:],
                        xt[:, :].rearrange("(a p) t -> p a t", p=128)[
                            :, :, nt * 512 : (nt + 1) * 512
                        ],
                    )
                    ps = ps_med.tile([128, 512], dt, tag="mm512")
                    for a in range(8):
                        nc.tensor.matmul(
                            ps[:],
                            w_sb[:, a, :],
                            x_t[:, a, :],
                            start=(a == 0),
                            stop=(a == 7),
                        )
                    nc.scalar.activation(
                        y_sb[:, nt * 512 : (nt + 1) * 512],
                        ps[:],
                        AF.Identity,
                        bias=b_sb[:],
                    )

            # ---- V projection (natural layout) ----
            for nt in range(T // 512):
                x_t = xpool.tile([128, 8, 512], dt, tag="xt")
                nc.sync.dma_start(
                    x_t[:],
                    xt_v[:, :].rearrange("(a p) t -> p a t", p=128)[
                        :, :, nt * 512 : (nt + 1) * 512
                    ],
                )
                for sub in range(4):
                    tt = nt * 4 + sub
                    ps = ps_med.tile([128, 512], dt, tag="mm512")
                    nc.tensor.matmul(
                        ps[:, :DLOC],
                        ones_row[:],
                        bv_sb[:],
                        start=True,
                        stop=False,
                    )
                    for a in range(8):
                        nc.tensor.matmul(
                            ps[:, :DLOC],
                            x_t[:, a, sub * 128 : (sub + 1) * 128],
                            wv_sb[:, a, :],
                            start=False,
                            stop=(a == 7),
                        )
                    nc.vector.tensor_copy(v_sb[:, tt, :], ps[:, :DLOC])

            # ---- attention ----
            evict_flip = 0
            for qb in range(NQB):
                bias_t = biasp.tile([128, S], dt)
                nc.sync.dma_start(bias_t[:], bias_m[qb * 128 : (qb + 1) * 128, :])
                for b in range(B):
                    ctx_pair = ctxp.tile([128, 128], dt)
                    for h in range(HC):
                        d0 = h * D
                        sc = ps_big.tile([128, S], dt, tag="scores")
                        for kn in range(NKN):
                            ksl = slice(kn * 512, (kn + 1) * 512)
                            nc.tensor.matmul(
                                sc[:, ksl],
                                ident[:],
                                bias_t[:, ksl],
                                start=True,
                                stop=False,
                            )
                            nc.tensor.matmul(
                                sc[:, ksl],
                                qt_sb[
                                    d0 : d0 + D,
                                    b * S + qb * 128 : b * S + (qb + 1) * 128,
                                ],
                                kt_sb[d0 : d0 + D, b * S + kn * 512 : b * S + (kn + 1) * 512],
                                start=False,
                                stop=True,
                            )
                        p_t = attnp.tile([128, S], dt)
                        den = denp.tile([128, 1], dt, tag="den")
                        nc.scalar.activation(p_t[:], sc[:], AF.Exp, accum_out=den[:])
                        rec = denp.tile([128, 1], dt, tag="rec")
                        nc.vector.reciprocal(rec[:], den[:])
                        nc.vector.tensor_scalar_mul(p_t[:], p_t[:], rec[:])
                        nc.sync.dma_start(
                            attn_out[b, h, qb * 128 : (qb + 1) * 128, :], p_t[:]
                        )
                        # context accumulation
                        ctx = ps_ctx.tile([128, D], dt, tag="ctx")
                        for kg in range(4):
                            pt_ps = ps_med.tile([128, 512], dt, tag="mm512")
                            for j in range(4):
                                kt = kg * 4 + j
                                nc.tensor.transpose(
                                    pt_ps[:, j * 128 : (j + 1) * 128],
                                    p_t[:, kt * 128 : (kt + 1) * 128],
                                    ident[:],
                                )
                            pt_sb = ptp.tile([128, 512], dt)
                            if evict_flip % 2 == 0:
                                nc.scalar.copy(pt_sb[:], pt_ps[:])
                            else:
                                nc.vector.tensor_copy(pt_sb[:], pt_ps[:])
                            evict_flip += 1
                            for j in range(4):
                                kt = kg * 4 + j
                                nc.tensor.matmul(
                                    ctx[:],
                                    pt_sb[:, j * 128 : (j + 1) * 128],
                                    v_sb[:, b * NKT + kt, d0 : d0 + D],
                                    start=(kt == 0),
                                    stop=(kt == NKT - 1),
                                )
                        nc.vector.tensor_copy(ctx_pair[:, d0 : d0 + D], ctx[:])
                    # transpose both heads' context at once
                    ctT_ps = ps_med.tile([128, 128], dt, tag="mm512")
                    nc.tensor.transpose(ctT_ps[:], ctx_pair[:], ident[:])
                    nc.vector.tensor_copy(
                        ctxT[b][:, qb * 128 : (qb + 1) * 128], ctT_ps[:]
                    )

            # ---- output projection (partial; host reduces across cores) ----
            for b in range(B):
                for tt in range(S // 128):
                    o_row = outp.tile([128, E], dt)
                    for en in range(E // 512):
                        ps = ps_med.tile([128, 512], dt, tag="mm512")
                        nc.tensor.matmul(
                            ps[:],
                            ctxT[b][:, tt * 128 : (tt + 1) * 128],
                            wot_sb[:, en * 512 : (en + 1) * 512],
                            start=True,
                            stop=True,
                        )
                        if en % 2 == 0:
                            nc.scalar.copy(o_row[:, en * 512 : (en + 1) * 512], ps[:])
                        else:
                            nc.vector.tensor_copy(
                                o_row[:, en * 512 : (en + 1) * 512], ps[:]
                            )
                    t0 = b * S + tt * 128
                    nc.sync.dma_start(out_part[t0 : t0 + 128, :], o_row[:])

    nc.compile()
    return nc


def _get_nc():
    if "nc" not in _CACHE:
        _CACHE["nc"] = _build_nc()
    return _CACHE["nc"]


def kernel(query, key_, value, Wq, bq, Wk, bk, Wv, bv, Wo, bo, bias_matrix):
    from concourse.bass_utils import run_bass_kernel_spmd

    query = np.asarray(query, F32)
    key_ = np.asarray(key_, F32)
    value = np.asarray(value, F32)
    Wq = np.asarray(Wq, F32)
    bq = np.asarray(bq, F32)
    Wk = np.asarray(Wk, F32)
    bk = np.asarray(bk, F32)
    Wv = np.asarray(Wv, F32)
    bv = np.asarray(bv, F32)
    Wo = np.asarray(Wo, F32)
    bo = np.asarray(bo, F32)
    bias_matrix = np.asarray(bias_matrix, F32)

    # [S,B,E] -> X^T [E, T] with tokens b-major (t = b*S + s)
    xt_q = np.ascontiguousarray(query.transpose(2, 1, 0).reshape(E, T))
    xt_k = np.ascontiguousarray(key_.transpose(2, 1, 0).reshape(E, T))
    xt_v = np.ascontiguousarray(value.transpose(2, 1, 0).reshape(E, T))
    bias_m = np.ascontiguousarray(bias_matrix)

    scale = F32(D ** -0.5)
    WqT = np.ascontiguousarray(Wq.T) * scale  # [E_in, E_out], scaled
    WkT = np.ascontiguousarray(Wk.T)
    WvT = np.ascontiguousarray(Wv.T)
    WoT = np.ascontiguousarray(Wo.T)  # [E_in(=head dims), E_out]

    in_maps = []
    for c in range(N_CORES):
        rows = slice(c * DLOC, (c + 1) * DLOC)
        in_maps.append(
            {
                "xt_q": xt_q,
                "xt_k": xt_k,
                "xt_v": xt_v,
                "wt_q": np.ascontiguousarray(WqT[:, rows]),
                "wt_k": np.ascontiguousarray(WkT[:, rows]),
                "wt_v": np.ascontiguousarray(WvT[:, rows]),
                "wot": np.ascontiguousarray(WoT[rows, :]),
                "b_q": np.ascontiguousarray((bq[rows] * scale).reshape(DLOC, 1)),
                "b_k": np.ascontiguousarray(bk[rows].reshape(DLOC, 1)),
                "b_v": np.ascontiguousarray(bv[rows].reshape(1, DLOC)),
                "bias_m": bias_m,
            }
        )

    global last_in_maps
    last_in_maps = in_maps
    nc = _get_nc()
    res = run_bass_kernel_spmd(nc, in_maps, core_ids=list(range(N_CORES)))

    attn = np.empty((B, H, S, S), F32)
    out_t = np.zeros((T, E), F32)
    for c in range(N_CORES):
        attn[:, c * HC : (c + 1) * HC] = res.results[c]["attn_out"]
        out_t += res.results[c]["out_part"]
    out = out_t.reshape(B, S, E) + bo  # tokens are b-major
    out = np.ascontiguousarray(out.transpose(1, 0, 2))  # [S, B, E]
    return out, attn


# revision 7
# speedup vs baseline: 1.7466x; 1.7466x over previous
"""Trainium2 Bass kernel for CustomMultiheadAttention.

Shapes: query/key_/value [S=2048, B=2, E=1024] f32, H=16 heads, D=64.
Returns (out [S,B,E], attn [B,H,S,S]) like the torch/jax reference.

Sharding: tensor-parallel over heads — each of the 8 NeuronCores computes
2 heads' Q/K/V projections, full attention for its (batch, head) pairs,
its attention-probability output slice, and a partial output projection
(summed on host).

Perf notes (TRN2): plain fp32 matmul streams at 4 cycles/row; float32r
(TF32-like rounded fp32) streams at 1 cycle/row when the moving dim is
>=256, so every large matmul here uses float32r operands. The softmax
itself (exp + normalize, via ScalarE accum_out for the denominator) is
exact fp32, so attention probabilities keep ~1e-4 accuracy.
"""

import numpy as np

S, B, E, H = 2048, 2, 1024, 16
D = E // H  # 64
N_CORES = 8
HC = H // N_CORES  # heads per core = 2
DLOC = HC * D  # 128
T = S * B  # 4096 tokens, b-major (t = b*S + s)

F32 = np.float32

_CACHE = {}


def _build_nc():
    import concourse.bacc as bacc
    import concourse.mybir as mybir
    import concourse.tile as tile
    from concourse.masks import make_identity

    dt = mybir.dt.float32
    dtr = mybir.dt.float32r
    dtb = mybir.dt.bfloat16
    AF = mybir.ActivationFunctionType

    nc = bacc.Bacc(None, target_bir_lowering=False, debug=False)

    xt_q = nc.dram_tensor("xt_q", [E, T], dt, kind="ExternalInput")
    xt_k = nc.dram_tensor("xt_k", [E, T], dt, kind="ExternalInput")
    xt_v = nc.dram_tensor("xt_v", [E, T], dt, kind="ExternalInput")
    wt_q = nc.dram_tensor("wt_q", [E, DLOC], dt, kind="ExternalInput")
    wt_k = nc.dram_tensor("wt_k", [E, DLOC], dt, kind="ExternalInput")
    wt_v = nc.dram_tensor("wt_v", [E, DLOC], dt, kind="ExternalInput")
    wot = nc.dram_tensor("wot", [DLOC, E], dt, kind="ExternalInput")
    b_q = nc.dram_tensor("b_q", [DLOC, 1], dt, kind="ExternalInput")
    b_k = nc.dram_tensor("b_k", [DLOC, 1], dt, kind="ExternalInput")
    b_v = nc.dram_tensor("b_v", [DLOC, 1], dt, kind="ExternalInput")
    bias_m = nc.dram_tensor("bias_m", [S, S], dtb, kind="ExternalInput")

    attn_out = nc.dram_tensor("attn_out", [B, HC, S, S], dt, kind="ExternalOutput")
    out_part = nc.dram_tensor("out_part", [T, E], dt, kind="ExternalOutput")

    NQG = 4  # q groups of 512 rows
    NKT = S // 128  # 16 k tiles per batch

    with tile.TileContext(nc) as tc:
        with (
            tc.tile_pool(name="const", bufs=1) as const,
            tc.tile_pool(name="wpool", bufs=1) as wpool,
            tc.tile_pool(name="xpool", bufs=2) as xpool,
            tc.tile_pool(name="act", bufs=1) as act,
            tc.tile_pool(name="vtp", bufs=2) as vtp,
            tc.tile_pool(name="biasp", bufs=2) as biasp,
            tc.tile_pool(name="attnp", bufs=5) as attnp,
            tc.tile_pool(name="ptp", bufs=3) as ptp,
            tc.tile_pool(name="denp", bufs=8) as denp,
            tc.tile_pool(name="outp", bufs=2) as outp,
            tc.tile_pool(name="ps_big", bufs=1, space="PSUM") as ps_big,
            tc.tile_pool(name="ps_med", bufs=2, space="PSUM") as ps_med,
            tc.tile_pool(name="ps_ctx", bufs=2, space="PSUM") as ps_ctx,
        ):
            # ---- constants ----
            ident = const.tile([128, 128], dt)
            make_identity(nc, ident[:])
            ident_r = const.tile([128, 128], dtr)
            nc.vector.tensor_copy(ident_r[:], ident[:])
            ident_b = const.tile([128, 128], dtb)
            nc.vector.tensor_copy(ident_b[:], ident[:])

            # ---- weights (cast to f32r on load) ----
            wq_sb = wpool.tile([128, 8, DLOC], dtr)
            wk_sb = wpool.tile([128, 8, DLOC], dtr)
            wv_sb = wpool.tile([128, 8, DLOC], dtr)
            nc.gpsimd.dma_start(wq_sb[:], wt_q[:, :].rearrange("(a p) m -> p a m", p=128))
            nc.gpsimd.dma_start(wk_sb[:], wt_k[:, :].rearrange("(a p) m -> p a m", p=128))
            nc.gpsimd.dma_start(wv_sb[:], wt_v[:, :].rearrange("(a p) m -> p a m", p=128))
            wot_sb = wpool.tile([128, E], dtr)
            nc.gpsimd.dma_start(wot_sb[:], wot[:, :])
            bq_sb = wpool.tile([DLOC, 1], dt)
            bk_sb = wpool.tile([DLOC, 1], dt)
            bv_sb = wpool.tile([DLOC, 1], dt)
            nc.sync.dma_start(bq_sb[:], b_q[:, :])
            nc.sync.dma_start(bk_sb[:], b_k[:, :])
            nc.sync.dma_start(bv_sb[:], b_v[:, :])

            # ---- persistent activations ----
            qt_sb = act.tile([128, T], dtr)  # Q^T  [dloc, token]
            kt_sb = act.tile([128, T], dtr)  # K^T  [dloc, token]
            v_sb = act.tile([128, T // 128, DLOC], dtr)  # V [token, dloc]
            ctxT0 = act.tile([128, S], dtr)  # context^T for batch 0
            ctxT1 = act.tile([128, S], dtr)
            ctxT = [ctxT0, ctxT1]

            # ---- Q, K, V projections (Y^T layout [dloc, token]) ----
            for name, xt, w_sb, b_sb in (
                ("q", xt_q, wq_sb, bq_sb),
                ("k", xt_k, wk_sb, bk_sb),
                ("v", xt_v, wv_sb, bv_sb),
            ):
                for nt in range(T // 512):
                    x_t = xpool.tile([128, 8, 512], dtr, tag="xt")
                    nc.gpsimd.dma_start(
                        x_t[:],
                        xt[:, :].rearrange("(a p) t -> p a t", p=128)[
                            :, :, nt * 512 : (nt + 1) * 512
                        ],
                    )
                    ps = ps_med.tile([128, 512], dt, tag="mm512")
                    for a in range(8):
                        nc.tensor.matmul(
                            ps[:],
                            w_sb[:, a, :],
                            x_t[:, a, :],
                            start=(a == 0),
                            stop=(a == 7),
                        )
                    if name != "v":
                        y_sb = qt_sb if name == "q" else kt_sb
                        nc.scalar.activation(
                            y_sb[:, nt * 512 : (nt + 1) * 512],
                            ps[:],
                            AF.Identity,
                            bias=b_sb[:],
                        )
                    else:
                        vt_t = vtp.tile([128, 512], dtr)
                        nc.scalar.activation(
                            vt_t[:], ps[:], AF.Identity, bias=b_sb[:]
                        )
                        # transpose to natural V [token, dloc]
                        for sub in range(4):
                            tt = nt * 4 + sub
                            tps = ps_med.tile([128, 128], dtr, tag="mm512")
                            nc.tensor.transpose(
                                tps[:],
                                vt_t[:, sub * 128 : (sub + 1) * 128],
                                ident_r[:],
                            )
                            nc.vector.tensor_copy(v_sb[:, tt, :], tps[:])

            # ---- attention ----
            flip = 0
            bias_tiles = {}
            for qg in range(NQG):
                for b in range(B):
                    for h in range(HC):
                        d0 = h * D
                        p_ts = []
                        for qj in range(4):
                            qb = qg * 4 + qj
                            if b == 0 and h == 0:
                                bias_t = biasp.tile([128, S], dtb, tag=f"bias{qj}")
                                nc.sync.dma_start(
                                    bias_t[:], bias_m[qb * 128 : (qb + 1) * 128, :]
                                )
                                bias_tiles[qb] = bias_t
                            else:
                                bias_t = bias_tiles[qb]
                            sc = ps_big.tile([128, S], dt, tag="scores")
                            for kn in range(4):
                                ksl = slice(kn * 512, (kn + 1) * 512)
                                nc.tensor.matmul(
                                    sc[:, ksl],
                                    ident_b[:],
                                    bias_t[:, ksl],
                                    start=True,
                                    stop=False,
                                )
                                nc.tensor.matmul(
                                    sc[:, ksl],
                                    qt_sb[
                                        d0 : d0 + D,
                                        b * S + qb * 128 : b * S + (qb + 1) * 128,
                                    ],
                                    kt_sb[
                                        d0 : d0 + D,
                                        b * S + kn * 512 : b * S + (kn + 1) * 512,
                                    ],
                                    start=False,
                                    stop=True,
                                )
                            p_t = attnp.tile([128, S], dt)
                            den = denp.tile([128, 1], dt, tag="den")
                            nc.scalar.activation(
                                p_t[:], sc[:], AF.Exp, accum_out=den[:]
                            )
                            rec = denp.tile([128, 1], dt, tag="rec")
                            nc.vector.reciprocal(rec[:], den[:])
                            nc.vector.tensor_scalar_mul(p_t[:], p_t[:], rec[:])
                            nc.sync.dma_start(
                                attn_out[b, h, qb * 128 : (qb + 1) * 128, :], p_t[:]
                            )
                            p_ts.append(p_t)
                        # context^T [64, 512] over this q-group
                        ctx = ps_ctx.tile([64, 512], dt, tag="ctx")
                        for kt in range(NKT):
                            pt_ps = ps_med.tile([128, 512], dt, tag="mm512")
                            for qj in range(4):
                                nc.tensor.transpose(
                                    pt_ps[:, qj * 128 : (qj + 1) * 128],
                                    p_ts[qj][:, kt * 128 : (kt + 1) * 128],
                                    ident[:],
                                )
                            pt_sb = ptp.tile([128, 512], dtr)
                            if flip % 2 == 0:
                                nc.scalar.copy(pt_sb[:], pt_ps[:])
                            else:
                                nc.vector.tensor_copy(pt_sb[:], pt_ps[:])
                            flip += 1
                            nc.tensor.matmul(
                                ctx[:],
                                v_sb[:, b * NKT + kt, d0 : d0 + D],
                                pt_sb[:],
                                start=(kt == 0),
                                stop=(kt == NKT - 1),
                            )
                        nc.vector.tensor_copy(
                            ctxT[b][d0 : d0 + D, qg * 512 : (qg + 1) * 512], ctx[:]
                        )

            # ---- output projection (partial; host reduces across cores) ----
            for b in range(B):
                for tt in range(S // 128):
                    o_row = outp.tile([128, E], dt)
                    for en in range(E // 512):
                        ps = ps_med.tile([128, 512], dt, tag="mm512")
                        nc.tensor.matmul(
                            ps[:],
                            ctxT[b][:, tt * 128 : (tt + 1) * 128],
                            wot_sb[:, en * 512 : (en + 1) * 512],
                            start=True,
                            stop=True,
                        )
                        if en % 2 == 0:
                            nc.scalar.copy(o_row[:, en * 512 : (en + 1) * 512], ps[:])
                        else:
                            nc.vector.tensor_copy(
                                o_row[:, en * 512 : (en + 1) * 512], ps[:]
                            )
                    t0 = b * S + tt * 128
                    nc.sync.dma_start(out_part[t0 : t0 + 128, :], o_row[:])

    nc.compile()
    return nc


def _get_nc():
    if "nc" not in _CACHE:
        _CACHE["nc"] = _build_nc()
    return _CACHE["nc"]


def kernel(query, key_, value, Wq, bq, Wk, bk, Wv, bv, Wo, bo, bias_matrix):
    import ml_dtypes
    from concourse.bass_utils import run_bass_kernel_spmd

    query = np.asarray(query, F32)
    key_ = np.asarray(key_, F32)
    value = np.asarray(value, F32)
    Wq = np.asarray(Wq, F32)
    bq = np.asarray(bq, F32)
    Wk = np.asarray(Wk, F32)
    bk = np.asarray(bk, F32)
    Wv = np.asarray(Wv, F32)
    bv = np.asarray(bv, F32)
    Wo = np.asarray(Wo, F32)
    bo = np.asarray(bo, F32)
    bias_matrix = np.asarray(bias_matrix, F32)

    # [S,B,E] -> X^T [E, T] with tokens b-major (t = b*S + s)
    xt_q = np.ascontiguousarray(query.transpose(2, 1, 0).reshape(E, T))
    xt_k = np.ascontiguousarray(key_.transpose(2, 1, 0).reshape(E, T))
    xt_v = np.ascontiguousarray(value.transpose(2, 1, 0).reshape(E, T))
    bias_m = np.ascontiguousarray(bias_matrix.astype(ml_dtypes.bfloat16))

    scale = F32(D ** -0.5)
    WqT = np.ascontiguousarray(Wq.T) * scale  # [E_in, E_out], scaled
    WkT = np.ascontiguousarray(Wk.T)
    WvT = np.ascontiguousarray(Wv.T)
    WoT = np.ascontiguousarray(Wo.T)  # [E_in(=head dims), E_out]

    in_maps = []
    for c in range(N_CORES):
        rows = slice(c * DLOC, (c + 1) * DLOC)
        in_maps.append(
            {
                "xt_q": xt_q,
                "xt_k": xt_k,
                "xt_v": xt_v,
                "wt_q": np.ascontiguousarray(WqT[:, rows]),
                "wt_k": np.ascontiguousarray(WkT[:, rows]),
                "wt_v": np.ascontiguousarray(WvT[:, rows]),
                "wot": np.ascontiguousarray(WoT[rows, :]),
                "b_q": np.ascontiguousarray((bq[rows] * scale).reshape(DLOC, 1)),
                "b_k": np.ascontiguousarray(bk[rows].reshape(DLOC, 1)),
                "b_v": np.ascontiguousarray(bv[rows].reshape(DLOC, 1)),
                "bias_m": bias_m,
            }
        )

    global last_in_maps
    last_in_maps = in_maps
    nc = _get_nc()
    res = run_bass_kernel_spmd(nc, in_maps, core_ids=list(range(N_CORES)))

    attn = np.empty((B, H, S, S), F32)
    out_t = np.zeros((T, E), F32)
    for c in range(N_CORES):
        attn[:, c * HC : (c + 1) * HC] = res.results[c]["attn_out"]
        out_t += res.results[c]["out_part"]
    out = out_t.reshape(B, S, E) + bo  # tokens are b-major
    out = np.ascontiguousarray(out.transpose(1, 0, 2))  # [S, B, E]
    return out, attn


# revision 12
# speedup vs baseline: 2.0788x; 1.1902x over previous
"""Trainium2 Bass kernel for CustomMultiheadAttention.

Shapes: query/key_/value [S=2048, B=2, E=1024] f32, H=16 heads, D=64.
Returns (out [S,B,E], attn [B,H,S,S]) like the torch/jax reference.

Sharding: tensor-parallel over heads — each of the 8 NeuronCores computes
2 heads' Q/K/V projections, full attention for its (batch, head) pairs,
its attention-probability output slice, and a partial output projection
(summed on host).

Perf notes (TRN2): plain fp32 matmul streams at 4 cycles/row; float32r
(TF32-like rounded fp32) streams at 1 cycle/row when the moving dim is
>=256, so every large matmul here uses float32r operands. The softmax
itself (exp + normalize, via ScalarE accum_out for the denominator) is
exact fp32, so attention probabilities keep ~1e-4 accuracy.
"""

import numpy as np

S, B, E, H = 2048, 2, 1024, 16
D = E // H  # 64
N_CORES = 8
HC = H // N_CORES  # heads per core = 2
DLOC = HC * D  # 128
T = S * B  # 4096 tokens, b-major (t = b*S + s)

F32 = np.float32

_CACHE = {}


def _build_nc():
    import concourse.bacc as bacc
    import concourse.mybir as mybir
    import concourse.tile as tile
    from concourse.masks import make_identity

    dt = mybir.dt.float32
    dtr = mybir.dt.float32r
    dtb = mybir.dt.bfloat16
    AF = mybir.ActivationFunctionType

    nc = bacc.Bacc(None, target_bir_lowering=False, debug=False)

    xt_q = nc.dram_tensor("xt_q", [E, T], dt, kind="ExternalInput")
    xt_k = nc.dram_tensor("xt_k", [E, T], dt, kind="ExternalInput")
    xt_v = nc.dram_tensor("xt_v", [E, T], dt, kind="ExternalInput")
    wt_q = nc.dram_tensor("wt_q", [E, DLOC], dt, kind="ExternalInput")
    wt_k = nc.dram_tensor("wt_k", [E, DLOC], dt, kind="ExternalInput")
    wt_v = nc.dram_tensor("wt_v", [E, DLOC], dt, kind="ExternalInput")
    wot = nc.dram_tensor("wot", [DLOC, E], dt, kind="ExternalInput")
    b_q = nc.dram_tensor("b_q", [DLOC, 1], dt, kind="ExternalInput")
    b_k = nc.dram_tensor("b_k", [DLOC, 1], dt, kind="ExternalInput")
    b_v = nc.dram_tensor("b_v", [DLOC, 1], dt, kind="ExternalInput")
    bias_m = nc.dram_tensor("bias_m", [S, S], dtb, kind="ExternalInput")

    attn_out = nc.dram_tensor("attn_out", [B, HC, S, S], dtr, kind="ExternalOutput")
    out_part = nc.dram_tensor("out_part", [T, E], dt, kind="ExternalOutput")

    NQG = 4  # q groups of 512 rows
    NKT = S // 128  # 16 k tiles per batch

    with tile.TileContext(nc) as tc:
        with (
            tc.tile_pool(name="const", bufs=1) as const,
            tc.tile_pool(name="wpool", bufs=1) as wpool,
            tc.tile_pool(name="xpool", bufs=2) as xpool,
            tc.tile_pool(name="act", bufs=1) as act,
            tc.tile_pool(name="vtp", bufs=2) as vtp,
            tc.tile_pool(name="biasp", bufs=2) as biasp,
            tc.tile_pool(name="attnp", bufs=5) as attnp,
            tc.tile_pool(name="ptp", bufs=3) as ptp,
            tc.tile_pool(name="denp", bufs=8) as denp,
            tc.tile_pool(name="outp", bufs=2) as outp,
            tc.tile_pool(name="ps_big", bufs=2, space="PSUM") as ps_big,
            tc.tile_pool(name="ps_med", bufs=2, space="PSUM") as ps_med,
            tc.tile_pool(name="ps_ctx", bufs=2, space="PSUM") as ps_ctx,
        ):
            # ---- constants ----
            ident = const.tile([128, 128], dt)
            make_identity(nc, ident[:])
            ident_r = const.tile([128, 128], dtr)
            nc.vector.tensor_copy(ident_r[:], ident[:])
            ident_b = const.tile([128, 128], dtb)
            nc.vector.tensor_copy(ident_b[:], ident[:])

            # ---- weights (cast to f32r on load) ----
            wq_sb = wpool.tile([128, 8, DLOC], dtr)
            wk_sb = wpool.tile([128, 8, DLOC], dtr)
            wv_sb = wpool.tile([128, 8, DLOC], dtr)
            nc.gpsimd.dma_start(wq_sb[:], wt_q[:, :].rearrange("(a p) m -> p a m", p=128))
            nc.gpsimd.dma_start(wk_sb[:], wt_k[:, :].rearrange("(a p) m -> p a m", p=128))
            nc.gpsimd.dma_start(wv_sb[:], wt_v[:, :].rearrange("(a p) m -> p a m", p=128))
            wot_sb = wpool.tile([128, E], dtr)
            nc.gpsimd.dma_start(wot_sb[:], wot[:, :])
            bq_sb = wpool.tile([DLOC, 1], dt)
            bk_sb = wpool.tile([DLOC, 1], dt)
            bv_sb = wpool.tile([DLOC, 1], dt)
            nc.sync.dma_start(bq_sb[:], b_q[:, :])
            nc.sync.dma_start(bk_sb[:], b_k[:, :])
            nc.sync.dma_start(bv_sb[:], b_v[:, :])

            # ---- persistent activations ----
            qt_sb = act.tile([128, T], dtr)  # Q^T  [dloc, token]
            kt_sb = act.tile([128, T], dtr)  # K^T  [dloc, token]
            v_sb = act.tile([128, T // 128, DLOC], dtr)  # V [token, dloc]
            ctxT0 = act.tile([128, S], dtr)  # context^T for batch 0
            ctxT1 = act.tile([128, S], dtr)
            ctxT = [ctxT0, ctxT1]

            # ---- Q, K, V projections (Y^T layout [dloc, token]) ----
            for name, xt, w_sb, b_sb in (
                ("q", xt_q, wq_sb, bq_sb),
                ("k", xt_k, wk_sb, bk_sb),
                ("v", xt_v, wv_sb, bv_sb),
            ):
                for nt in range(T // 512):
                    x_t = xpool.tile([128, 8, 512], dtr, tag="xt")
                    nc.gpsimd.dma_start(
                        x_t[:],
                        xt[:, :].rearrange("(a p) t -> p a t", p=128)[
                            :, :, nt * 512 : (nt + 1) * 512
                        ],
                    )
                    ps = ps_med.tile([128, 512], dt, tag="mm512")
                    for a in range(8):
                        nc.tensor.matmul(
                            ps[:],
                            w_sb[:, a, :],
                            x_t[:, a, :],
                            start=(a == 0),
                            stop=(a == 7),
                        )
                    if name != "v":
                        y_sb = qt_sb if name == "q" else kt_sb
                        nc.scalar.activation(
                            y_sb[:, nt * 512 : (nt + 1) * 512],
                            ps[:],
                            AF.Identity,
                            bias=b_sb[:],
                        )
                    else:
                        vt_t = vtp.tile([128, 512], dtr)
                        nc.scalar.activation(
                            vt_t[:], ps[:], AF.Identity, bias=b_sb[:]
                        )
                        # transpose to natural V [token, dloc]
                        for sub in range(4):
                            tt = nt * 4 + sub
                            tps = ps_med.tile([128, 128], dtr, tag="mm512")
                            nc.tensor.transpose(
                                tps[:],
                                vt_t[:, sub * 128 : (sub + 1) * 128],
                                ident_r[:],
                            )
                            nc.vector.tensor_copy(v_sb[:, tt, :], tps[:])

            # ---- attention ----
            flip = 0
            bias_tiles = {}
            for qg in range(NQG):
                for b in range(B):
                    for h in range(HC):
                        d0 = h * D
                        p_ts = []
                        for qj in range(4):
                            qb = qg * 4 + qj
                            if b == 0 and h == 0:
                                bias_t = biasp.tile([128, S], dtb, tag=f"bias{qj}")
                                nc.sync.dma_start(
                                    bias_t[:], bias_m[qb * 128 : (qb + 1) * 128, :]
                                )
                                bias_tiles[qb] = bias_t
                            else:
                                bias_t = bias_tiles[qb]
                            p_t = attnp.tile([128, S], dtr)
                            dens = []
                            for kh in range(2):
                                sc = ps_big.tile([128, 1024], dt, tag="scores")
                                for kn2 in range(2):
                                    kn = kh * 2 + kn2
                                    ksl = slice(kn2 * 512, (kn2 + 1) * 512)
                                    nc.tensor.matmul(
                                        sc[:, ksl],
                                        ident_b[:],
                                        bias_t[:, kn * 512 : (kn + 1) * 512],
                                        start=True,
                                        stop=False,
                                    )
                                    nc.tensor.matmul(
                                        sc[:, ksl],
                                        qt_sb[
                                            d0 : d0 + D,
                                            b * S + qb * 128 : b * S + (qb + 1) * 128,
                                        ],
                                        kt_sb[
                                            d0 : d0 + D,
                                            b * S + kn * 512 : b * S + (kn + 1) * 512,
                                        ],
                                        start=False,
                                        stop=True,
                                    )
                                den_h = denp.tile([128, 1], dt, tag=f"den{kh}")
                                nc.scalar.activation(
                                    p_t[:, kh * 1024 : (kh + 1) * 1024],
                                    sc[:],
                                    AF.Exp,
                                    accum_out=den_h[:],
                                )
                                dens.append(den_h)
                            rec = denp.tile([128, 1], dt, tag="rec")
                            nc.vector.tensor_add(rec[:], dens[0][:], dens[1][:])
                            nc.vector.reciprocal(rec[:], rec[:])
                            nc.vector.tensor_scalar_mul(p_t[:], p_t[:], rec[:])
                            nc.sync.dma_start(
                                attn_out[b, h, qb * 128 : (qb + 1) * 128, :], p_t[:]
                            )
                            p_ts.append(p_t)
                        # context^T [64, 512] over this q-group
                        ctx = ps_ctx.tile([64, 512], dt, tag="ctx")
                        for kt in range(NKT):
                            pt_ps = ps_med.tile([128, 512], dtr, tag="mm512")
                            for qj in range(4):
                                nc.tensor.transpose(
                                    pt_ps[:, qj * 128 : (qj + 1) * 128],
                                    p_ts[qj][:, kt * 128 : (kt + 1) * 128],
                                    ident_r[:],
                                )
                            pt_sb = ptp.tile([128, 512], dtr)
                            if flip % 2 == 0:
                                nc.scalar.copy(pt_sb[:], pt_ps[:])
                            else:
                                nc.vector.tensor_copy(pt_sb[:], pt_ps[:])
                            flip += 1
                            nc.tensor.matmul(
                                ctx[:],
                                v_sb[:, b * NKT + kt, d0 : d0 + D],
                                pt_sb[:],
                                start=(kt == 0),
                                stop=(kt == NKT - 1),
                            )
                        nc.vector.tensor_copy(
                            ctxT[b][d0 : d0 + D, qg * 512 : (qg + 1) * 512], ctx[:]
                        )

            # ---- output projection (partial; host reduces across cores) ----
            for b in range(B):
                for tt in range(S // 128):
                    o_row = outp.tile([128, E], dt)
                    for en in range(E // 512):
                        ps = ps_med.tile([128, 512], dt, tag="mm512")
                        nc.tensor.matmul(
                            ps[:],
                            ctxT[b][:, tt * 128 : (tt + 1) * 128],
                            wot_sb[:, en * 512 : (en + 1) * 512],
                            start=True,
                            stop=True,
                        )
                        nc.vector.tensor_copy(
                            o_row[:, en * 512 : (en + 1) * 512], ps[:]
                        )
                    t0 = b * S + tt * 128
                    nc.sync.dma_start(out_part[t0 : t0 + 128, :], o_row[:])

    nc.compile()
    return nc


def _get_nc():
    if "nc" not in _CACHE:
        _CACHE["nc"] = _build_nc()
    return _CACHE["nc"]


def kernel(query, key_, value, Wq, bq, Wk, bk, Wv, bv, Wo, bo, bias_matrix):
    import ml_dtypes
    from concourse.bass_utils import run_bass_kernel_spmd

    query = np.asarray(query, F32)
    key_ = np.asarray(key_, F32)
    value = np.asarray(value, F32)
    Wq = np.asarray(Wq, F32)
    bq = np.asarray(bq, F32)
    Wk = np.asarray(Wk, F32)
    bk = np.asarray(bk, F32)
    Wv = np.asarray(Wv, F32)
    bv = np.asarray(bv, F32)
    Wo = np.asarray(Wo, F32)
    bo = np.asarray(bo, F32)
    bias_matrix = np.asarray(bias_matrix, F32)

    # [S,B,E] -> X^T [E, T] with tokens b-major (t = b*S + s)
    xt_q = np.ascontiguousarray(query.transpose(2, 1, 0).reshape(E, T))
    xt_k = np.ascontiguousarray(key_.transpose(2, 1, 0).reshape(E, T))
    xt_v = np.ascontiguousarray(value.transpose(2, 1, 0).reshape(E, T))
    bias_m = np.ascontiguousarray(bias_matrix.astype(ml_dtypes.bfloat16))

    scale = F32(D ** -0.5)
    WqT = np.ascontiguousarray(Wq.T) * scale  # [E_in, E_out], scaled
    WkT = np.ascontiguousarray(Wk.T)
    WvT = np.ascontiguousarray(Wv.T)
    WoT = np.ascontiguousarray(Wo.T)  # [E_in(=head dims), E_out]

    in_maps = []
    for c in range(N_CORES):
        rows = slice(c * DLOC, (c + 1) * DLOC)
        in_maps.append(
            {
                "xt_q": xt_q,
                "xt_k": xt_k,
                "xt_v": xt_v,
                "wt_q": np.ascontiguousarray(WqT[:, rows]),
                "wt_k": np.ascontiguousarray(WkT[:, rows]),
                "wt_v": np.ascontiguousarray(WvT[:, rows]),
                "wot": np.ascontiguousarray(WoT[rows, :]),
                "b_q": np.ascontiguousarray((bq[rows] * scale).reshape(DLOC, 1)),
                "b_k": np.ascontiguousarray(bk[rows].reshape(DLOC, 1)),
                "b_v": np.ascontiguousarray(bv[rows].reshape(DLOC, 1)),
                "bias_m": bias_m,
            }
        )

    global last_in_maps
    last_in_maps = in_maps
    nc = _get_nc()
    res = run_bass_kernel_spmd(nc, in_maps, core_ids=list(range(N_CORES)))

    attn = np.empty((B, H, S, S), F32)
    out_t = np.zeros((T, E), F32)
    for c in range(N_CORES):
        attn[:, c * HC : (c + 1) * HC] = res.results[c]["attn_out"]
        out_t += res.results[c]["out_part"]
    out = out_t.reshape(B, S, E) + bo  # tokens are b-major
    out = np.ascontiguousarray(out.transpose(1, 0, 2))  # [S, B, E]
    return out, attn
